# revision 27
# baseline (speedup 1.0000x reference)
"""Trainium2 Bass kernel for nn_CBAMSpaceMask (CBAM spatial mask, T timestep blocks).

Math per timestep t (channels c=0..2, input planes q=3t+c):
  mx_c = maxpool3x3(x_q)                (-inf pad == clamp)
  av_c = boxsum3x3(x_q)/9               (zero pad, count_include_pad)
  y_t  = sum_c wM_c (*) mx_c + wA_c (*) av_c + b     (3x3 conv, zero pad)
  out[3t+c'] = sigmoid(leakyrelu(y_t))  for c' = 0..2

Device decomposition (1 batch per core, pure batch data-parallel):
  - max path: vertical 3-max from three row-shifted HBM cast-loads (X/U/D,
    partition-aligned DMA swizzle, bf16); horizontal 3-max via the shift-by-2
    parity trick (aligned 2x op + one misaligned op) - no partition-shifted
    SBUF copies (those run at ~26 GB/s).
  - avg path: fully folded into the PE as width-5 column kernels applied to X
    directly (vertical box+conv in the row operator, horizontal box merged
    into the column taps), with exact col-0/col-255 correction matmuls.
  - conv: banded-Toeplitz row-operator matmuls, K=128 rows on partitions,
    2 timesteps per matmul (N~510), 8 PSUM banks, weights reused across pairs.
  - rows 0..249: two 128-row chunks split in 24-plane halves; rows 250..255:
    packed 9-partition blocks (8 rows + clamp-dup) x 14+2 timesteps.
  - epilogue: batched ACT Lrelu(+bias) then Sigmoid broadcast-writes; output
    DMA is contiguous row-major bf16 to a DRAM scratch; host transposes/casts.
"""
import sys

sys.path.insert(0, "/opt/trn_rl_repo")

import numpy as np
import ml_dtypes
from contextlib import ExitStack

import concourse.bass as bass
import concourse.tile as tile
from concourse import bacc, mybir
from concourse.bass_utils import run_bass_kernel_spmd

F32 = mybir.dt.float32
BF16 = mybir.dt.bfloat16

B, CTOT, H, W = 8, 48, 256, 256
T = 16
N_CORES = 8
PGRP = 12            # planes per load/pool group
# big chunks: (m0, m1, r0) -> y rows [m0, m1), X rows [r0, r0+128)
BIG_CHUNKS = [(0, 126, 0), (126, 250, 124)]
# packed chunk: y rows [250, 256), X rows 248..255 + dup, 9 partitions per t
C2_M0 = 250
C2_R0 = 248
C2_TILES = [(0, 14), (14, 2)]  # (t0, nt)
NMAT_PER_CHUNK = 30  # per c: 5 avg + corr0 + corr255 + 3 max
NMATS = 90

_cache = {}


def _build_ops(conv_w):
    """Per-channel operator matrices in float64.

    Returns per c: (OP5[5], corr0, corr255, OPmax[3]) each HxH acting on rows:
      y[m, w] = sum_s OP5_s[m, :] @ X[:, w+s-2]  (+ corr at w=0 / w=255)
             + sum_kw OPmax_kw[m, :] @ MX[:, w+kw-1]
    """
    w = conv_w[0].astype(np.float64)  # [6, 3, 3]

    def S(j):
        return np.eye(H, k=j)

    Bv = S(-1) + S(0) + S(1)
    out = []
    for c in range(3):
        wM, wA = w[2 * c], w[2 * c + 1]
        OPmax = [sum(wM[kh, kw] * S(kh - 1) for kh in range(3)) for kw in range(3)]
        OPv = [(sum(wA[kh, kw] * S(kh - 1) for kh in range(3)) @ Bv) / 9.0
               for kw in range(3)]
        OP5 = [np.zeros((H, H)) for _ in range(5)]
        for kw in range(3):
            for j in range(3):
                OP5[kw + j] += OPv[kw]
        out.append((OP5, -OPv[0], -OPv[2], OPmax))
    return out


def _mat_list(ops_c):
    """Order mats per c: OP5 s=0..4, corr0, corr255, OPmax kw=0..2 (10 mats)."""
    OP5, corr0, corr255, OPmax = ops_c
    return list(OP5) + [corr0, corr255] + list(OPmax)


def _build_stack(conv_w):
    """lhsT stack [128, NMATS, 128] bf16.

    mats 0..29:  chunk0, lhsT[j, mm] = MAT[m0+mm, r0+j]
    mats 30..59: chunk1, same
    mats 60..89: packed chunk, block-diagonal over 14 t:
                 lhsT[9t+j, 6t+mm] = MAT[250+mm, 248+j] (j=8 -> 0)
    """
    ops = _build_ops(conv_w)
    stack = np.zeros((128, NMATS, 128), dtype=np.float64)
    for g, (m0, m1, r0) in enumerate(BIG_CHUNKS):
        M = m1 - m0
        for c in range(3):
            for k, MAT in enumerate(_mat_list(ops[c])):
                stack[:, 30 * g + 10 * c + k, :M] = MAT[m0:m1, r0:r0 + 128].T
    for c in range(3):
        for k, MAT in enumerate(_mat_list(ops[c])):
            m = np.zeros((126, 84))
            for t in range(14):
                for j in range(8):  # j=8 (dup/pooled-256) stays 0
                    m[9 * t + j, 6 * t:6 * t + 6] = MAT[C2_M0:C2_M0 + 6,
                                                        C2_R0 + j]
            stack[:126, 60 + 10 * c + k, :84] = m
    return stack.astype(ml_dtypes.bfloat16)


# (kind, shift): kinds 'x' (rhs = X) and 'mx' (rhs = MX); corr handled apart
MAT_SHIFTS = [('x', s - 2) for s in range(5)] + [('corr0', 0), ('corr255', 0)] \
    + [('mx', kw - 1) for kw in range(3)]


def _build_program():
    nc = bacc.Bacc("TRN2", target_bir_lowering=False, debug=False,
                   enable_asserts=False)
    # host-pre-transposed bf16 input: x_rm[r, ch, w]
    x_ap = nc.dram_tensor("x", [H, CTOT, W], BF16, kind="ExternalInput").ap()
    cst_ap = nc.dram_tensor("cst", [128, NMATS, 128], BF16,
                            kind="ExternalInput").ap()
    bias_ap = nc.dram_tensor("bias", [128, 1], F32, kind="ExternalInput").ap()
    # row-major scratch: outm[r, ch, w]; host transposes to [ch, r, w]
    outm_ap = nc.dram_tensor("outm", [H, CTOT, W], BF16,
                             kind="ExternalOutput").ap()
    out2_aps = [nc.dram_tensor(f"out2{i}", [6 * nt, 3, W], BF16,
                               kind="ExternalOutput").ap()
                for i, (t0, nt) in enumerate(C2_TILES)]

    MAXOP = mybir.AluOpType.max
    LRELU = mybir.ActivationFunctionType.Lrelu
    SIGM = mybir.ActivationFunctionType.Sigmoid

    with tile.TileContext(nc) as tc, ExitStack() as ctx:
        const_pool = ctx.enter_context(tc.tile_pool(name="const", bufs=1))
        xpool = ctx.enter_context(tc.tile_pool(name="xp", bufs=3))
        mxpool = ctx.enter_context(tc.tile_pool(name="mxp", bufs=2))
        p1pool = ctx.enter_context(tc.tile_pool(name="p1", bufs=2))
        bigpool = ctx.enter_context(tc.tile_pool(name="bg", bufs=1))
        epipool = ctx.enter_context(tc.tile_pool(name="epi", bufs=8))
        psum_pool = ctx.enter_context(tc.tile_pool(name="psum", bufs=8,
                                                   space="PSUM"))

        cst = const_pool.tile([128, NMATS, 128], BF16, tag="cst")
        nc.sync.dma_start(out=cst[:], in_=cst_ap)
        bias = const_pool.tile([128, 1], F32, tag="bias")
        nc.sync.dma_start(out=bias[:], in_=bias_ap)

        def hpool(vmax, np_, fd):
            """Horizontal 3-max of vmax -> mx, cols in-lane."""
            hb = p1pool.tile([128, PGRP, W], BF16, tag="hb")
            nc.vector.tensor_tensor(out=hb[0:np_, 0:fd, 0:W - 2],
                                    in0=vmax[0:np_, 0:fd, 0:W - 2],
                                    in1=vmax[0:np_, 0:fd, 2:W], op=MAXOP)
            mx = mxpool.tile([128, PGRP, W], BF16, tag="mx")
            nc.vector.tensor_tensor(out=mx[0:np_, 0:fd, 1:W - 1],
                                    in0=vmax[0:np_, 0:fd, 1:W - 1],
                                    in1=hb[0:np_, 0:fd, 0:W - 2], op=MAXOP)
            nc.vector.tensor_tensor(out=mx[0:np_, 0:fd, 0:1],
                                    in0=vmax[0:np_, 0:fd, 0:1],
                                    in1=vmax[0:np_, 0:fd, 1:2], op=MAXOP)
            nc.vector.tensor_tensor(out=mx[0:np_, 0:fd, W - 1:W],
                                    in0=vmax[0:np_, 0:fd, W - 2:W - 1],
                                    in1=vmax[0:np_, 0:fd, W - 1:W], op=MAXOP)
            return mx

        import os as _os0
        DBG_NH = int(_os0.environ.get("DBG_NH", "16"))    # groups to emit
        DBG_NOMM = _os0.environ.get("DBG_NOMM", "0") == "1"
        DBG_NOEPI = _os0.environ.get("DBG_NOEPI", "0") == "1"
        half_ctr = [0]

        # ---------------- big chunks ----------------
        def big_chunk(g):
            m0, m1, r0 = BIG_CHUNKS[g]
            M = m1 - m0
            big = bigpool.tile([128, CTOT, W], BF16, tag="big")
            epi_jobs = []
            for h in range(CTOT // PGRP):
                if half_ctr[0] >= DBG_NH:
                    break
                half_ctr[0] += 1
                pl = slice(PGRP * h, PGRP * (h + 1))
                # row-major bf16 input: every load is partition-contiguous
                # (1 descriptor per partition) - SWDGE issue is trivial
                X = xpool.tile([128, PGRP, W], BF16, tag="x")
                nc.gpsimd.dma_start(out=X[:], in_=x_ap[r0:r0 + 128, pl, :])
                U = p1pool.tile([128, PGRP, W], BF16, tag="u")
                nc.gpsimd.dma_start(out=U[:], in_=x_ap[r0 + 1:r0 + 129, pl, :])
                Df = p1pool.tile([128, PGRP, W], BF16, tag="df")
                if r0 == 0:
                    nc.gpsimd.dma_start(out=Df[1:128], in_=x_ap[0:127, pl, :])
                    nc.gpsimd.dma_start(out=Df[0:1], in_=x_ap[0:1, pl, :])
                else:
                    nc.gpsimd.dma_start(out=Df[:],
                                        in_=x_ap[r0 - 1:r0 + 127, pl, :])
                m2 = p1pool.tile([128, PGRP, W], BF16, tag="m2")
                nc.vector.tensor_tensor(out=m2[:], in0=X[:], in1=U[:],
                                        op=MAXOP)
                vmax = p1pool.tile([128, PGRP, W], BF16, tag="vm")
                nc.vector.tensor_tensor(out=vmax[:], in0=m2[:], in1=Df[:],
                                        op=MAXOP)
                mx = hpool(vmax, 128, PGRP)

                pairs = range(2 * h, 2 * h + 2)
                ps_tiles = {}
                for tp in pairs:
                    ps_tiles[tp] = psum_pool.tile([128, 2, W], F32, tag="ps",
                                                  name=f"ps_{g}_{tp}")

                if DBG_NOMM:
                    continue
                # emission: X-dependent mats first (PE can start before
                # pools finish), then mx-dependent; mat-outer, pair-inner
                order = [(c, k) for c in range(3) for k in range(7)] + \
                    [(c, k) for c in range(3) for k in range(7, 10)]
                for mi, (c, k) in enumerate(order):
                    kind, s = MAT_SHIFTS[k]
                    if True:
                        gmat = 30 * g + 10 * c + k
                        first = (mi == 0)
                        last = (mi == NMAT_PER_CHUNK - 1)
                        lhsT = cst[0:128, gmat, 0:M]
                        for tp in pairs:
                            ps = ps_tiles[tp]
                            lq = 6 * tp + c - PGRP * h
                            if kind == 'corr0':
                                nc.tensor.matmul(
                                    ps[0:M, :, 0:1], lhsT,
                                    X[0:128, lq:lq + 4:3, 0:1],
                                    start=first, stop=last)
                            elif kind == 'corr255':
                                nc.tensor.matmul(
                                    ps[0:M, :, W - 1:W], lhsT,
                                    X[0:128, lq:lq + 4:3, W - 1:W],
                                    start=first, stop=last)
                            else:
                                src = X if kind == 'x' else mx
                                lo = max(0, -s)
                                hi = W - max(0, s)
                                nc.tensor.matmul(
                                    ps[0:M, :, lo:hi], lhsT,
                                    src[0:128, lq:lq + 4:3, lo + s:hi + s],
                                    start=first, stop=last)
                epi_jobs.append((list(pairs), ps_tiles))

            import os as _os
            if DBG_NOEPI:
                epi_jobs.clear()
            if _os.environ.get("NOEPI_BATCH", "0") == "1":
                for pairs_l, ps_tiles in epi_jobs:
                    for tp in pairs_l:
                        v = epipool.tile([128, 2, W], BF16, tag="v",
                                         name=f"v_{g}_{tp}")
                        nc.scalar.activation(v[0:M], ps_tiles[tp][0:M], LRELU,
                                             bias=bias[0:M], scale=1.0,
                                             alpha=0.01)
                        for c3 in range(3):
                            q = 6 * tp + c3
                            nc.scalar.activation(big[0:M, q:q + 4:3, :],
                                                 v[0:M], SIGM)
            else:
                v_tiles = {}
                for pairs_l, ps_tiles in epi_jobs:
                    for tp in pairs_l:
                        v = epipool.tile([128, 2, W], BF16, tag="v",
                                         name=f"v_{g}_{tp}")
                        nc.scalar.activation(v[0:M], ps_tiles[tp][0:M], LRELU,
                                             bias=bias[0:M], scale=1.0,
                                             alpha=0.01)
                        v_tiles[tp] = v
                for pairs_l, _ in epi_jobs:
                    for tp in pairs_l:
                        for c3 in range(3):
                            q = 6 * tp + c3
                            nc.scalar.activation(big[0:M, q:q + 4:3, :],
                                                 v_tiles[tp][0:M], SIGM)
            if _os.environ.get("NOOUT", "0") != "1" and not DBG_NOEPI:
                nc.scalar.dma_start(out=outm_ap[m0:m1], in_=big[0:M])

        # ---------------- packed last-rows chunk ----------------
        def packed_tile(i):
            t0, nt = C2_TILES[i]
            npart = 9 * nt
            X2 = xpool.tile([128, 3, W], BF16, tag="x2")
            # finite-fill the tail partitions (quadrant-aligned base) before
            # the real loads overwrite [0:npart)
            nc.vector.tensor_copy(
                X2[:].rearrange("p c w -> p (c w)"),
                cst[0:128, 0:6, 0:128].rearrange("p a b -> p (a b)"))
            for t in range(nt):
                nc.gpsimd.dma_start(
                    out=X2[9 * t:9 * t + 8],
                    in_=x_ap[C2_R0:C2_R0 + 8,
                             3 * (t0 + t):3 * (t0 + t) + 3, :])
            dup = x_ap[H - 1, 3 * t0:3 * (t0 + nt), :].rearrange(
                "(t c) w -> t c w", t=nt)
            nc.gpsimd.dma_start(
                out=X2[0:126].rearrange("(t r) c w -> t r c w",
                                        t=14)[0:nt, 8:9].squeeze(1),
                in_=dup)
            
            Xs = p1pool.tile([128, PGRP, W], BF16, tag="hb")  # reuse slot
            nc.sync.dma_start(out=Xs[0:npart, 0:3], in_=X2[1:npart + 1])
            m2 = p1pool.tile([128, PGRP, W], BF16, tag="m2")
            nc.vector.tensor_tensor(out=m2[0:npart, 0:3], in0=X2[0:npart],
                                    in1=Xs[0:npart, 0:3], op=MAXOP)
            m2s = p1pool.tile([128, PGRP, W], BF16, tag="df")  # reuse slot
            nc.sync.dma_start(out=m2s[1:npart, 0:3], in_=m2[0:npart - 1, 0:3])
            nc.vector.tensor_copy(m2s[0:1, 0:3], m2[0:1, 0:3])
            vmax = p1pool.tile([128, PGRP, W], BF16, tag="vm")
            nc.vector.tensor_tensor(out=vmax[0:npart, 0:3], in0=m2[0:npart, 0:3],
                                    in1=m2s[0:npart, 0:3], op=MAXOP)
            mx2 = hpool(vmax, npart, 3)

            ps = psum_pool.tile([128, 2, W], F32, tag="ps", name=f"ps2_{i}")
            mi = 0
            for c in range(3):
                for k, (kind, s) in enumerate(MAT_SHIFTS):
                    gmat = 60 + 10 * c + k
                    lhsT = cst[0:npart, gmat, 0:6 * nt]
                    first = (mi == 0)
                    last = (mi == NMAT_PER_CHUNK - 1)
                    if kind == 'corr0':
                        nc.tensor.matmul(ps[0:6 * nt, 0, 0:1], lhsT,
                                         X2[0:npart, c, 0:1],
                                         start=first, stop=last)
                    elif kind == 'corr255':
                        nc.tensor.matmul(ps[0:6 * nt, 0, W - 1:W], lhsT,
                                         X2[0:npart, c, W - 1:W],
                                         start=first, stop=last)
                    else:
                        src = X2 if kind == 'x' else mx2
                        lo = max(0, -s)
                        hi = W - max(0, s)
                        nc.tensor.matmul(ps[0:6 * nt, 0, lo:hi], lhsT,
                                         src[0:npart, c, lo + s:hi + s],
                                         start=first, stop=last)
                    mi += 1

            v = epipool.tile([128, 2, W], BF16, tag="v", name=f"v2_{i}")
            nc.scalar.activation(v[0:6 * nt, 0], ps[0:6 * nt, 0], LRELU,
                                 bias=bias[0:6 * nt], scale=1.0, alpha=0.01)
            big2 = bigpool.tile([128, 3, W], BF16, tag="big2")
            for c3 in range(3):
                nc.scalar.activation(big2[0:6 * nt, c3, :], v[0:6 * nt, 0], SIGM)
            nc.scalar.dma_start(out=out2_aps[i], in_=big2[0:6 * nt])

        import os
        if os.environ.get("C2MID", "1") == "1":
            big_chunk(0)
            packed_tile(0)
            packed_tile(1)
            big_chunk(1)
        else:
            import os as _os
            big_chunk(0)
            big_chunk(1)
            if _os.environ.get("SKIP_C2", "0") != "1":
                packed_tile(0)
                packed_tile(1)

    nc.compile()
    return nc


def _assemble(res):
    outs = []
    for i in range(N_CORES):
        om = np.asarray(res.results[i]["outm"]).astype(np.float32)
        o = np.ascontiguousarray(om.transpose(1, 0, 2))  # [ch, r, w]
        for ti, (t0, nt) in enumerate(C2_TILES):
            o2 = np.asarray(res.results[i][f"out2{ti}"]).astype(np.float32)
            for t in range(nt):
                for c3 in range(3):
                    o[3 * (t0 + t) + c3, C2_M0:C2_M0 + 6, :] = \
                        o2[6 * t:6 * t + 6, c3]
        outs.append(o)
    return np.stack(outs, axis=0)


def _prep_x(input_tensor):
    """[B, CTOT, H, W] f32 -> per-core row-major bf16 [H, CTOT, W]."""
    xb = np.asarray(input_tensor, dtype=np.float32).astype(ml_dtypes.bfloat16)
    xrm = np.ascontiguousarray(xb.transpose(0, 2, 1, 3))
    return xrm


def kernel(input_tensor, conv_w, conv_b):
    conv_w = np.asarray(conv_w, dtype=np.float32)
    conv_b = np.asarray(conv_b, dtype=np.float32)

    if "nc" not in _cache:
        _cache["nc"] = _build_program()
    nc = _cache["nc"]

    xrm = _prep_x(input_tensor)
    stack = _build_stack(conv_w)
    bias_vec = np.full((128, 1), conv_b[0], dtype=np.float32)
    in_maps = [
        {"x": xrm[i], "cst": stack, "bias": bias_vec}
        for i in range(N_CORES)
    ]
    res = run_bass_kernel_spmd(nc, in_maps, list(range(N_CORES)))
    return _assemble(res)


if __name__ == "__main__":
    rng = np.random.default_rng(0)
    x = rng.standard_normal((B, CTOT, H, W), dtype=np.float32)
    cw = rng.uniform(-0.1, 0.1, (1, 6, 3, 3)).astype(np.float32)
    cb = np.array([0.01], dtype=np.float32)
    o = kernel(x, cw, cb)
    print(o.shape, o.dtype)


# revision 28
# speedup vs baseline: 1.1283x; 1.1283x over previous
"""Trainium2 Bass kernel for nn_CBAMSpaceMask (CBAM spatial mask, T timestep blocks).

Math per timestep t (channels c=0..2, input planes q=3t+c):
  mx_c = maxpool3x3(x_q)                (-inf pad == clamp)
  av_c = boxsum3x3(x_q)/9               (zero pad, count_include_pad)
  y_t  = sum_c wM_c (*) mx_c + wA_c (*) av_c + b     (3x3 conv, zero pad)
  out[3t+c'] = sigmoid(leakyrelu(y_t))  for c' = 0..2

Device decomposition (1 batch per core, pure batch data-parallel):
  - max path: vertical 3-max from three row-shifted HBM cast-loads (X/U/D,
    partition-aligned DMA swizzle, bf16); horizontal 3-max via the shift-by-2
    parity trick (aligned 2x op + one misaligned op) - no partition-shifted
    SBUF copies (those run at ~26 GB/s).
  - avg path: fully folded into the PE as width-5 column kernels applied to X
    directly (vertical box+conv in the row operator, horizontal box merged
    into the column taps), with exact col-0/col-255 correction matmuls.
  - conv: banded-Toeplitz row-operator matmuls, K=128 rows on partitions,
    2 timesteps per matmul (N~510), 8 PSUM banks, weights reused across pairs.
  - rows 0..249: two 128-row chunks split in 24-plane halves; rows 250..255:
    packed 9-partition blocks (8 rows + clamp-dup) x 14+2 timesteps.
  - epilogue: batched ACT Lrelu(+bias) then Sigmoid broadcast-writes; output
    DMA is contiguous row-major bf16 to a DRAM scratch; host transposes/casts.
"""
import sys

sys.path.insert(0, "/opt/trn_rl_repo")

import numpy as np
import ml_dtypes
from contextlib import ExitStack

import concourse.bass as bass
import concourse.tile as tile
from concourse import bacc, mybir
from concourse.bass_utils import run_bass_kernel_spmd

F32 = mybir.dt.float32
BF16 = mybir.dt.bfloat16

B, CTOT, H, W = 8, 48, 256, 256
T = 16
N_CORES = 8
PGRP = 12            # planes per load/pool group
# big chunks: (m0, m1, r0) -> y rows [m0, m1), X rows [r0, r0+128)
BIG_CHUNKS = [(0, 126, 0), (126, 250, 124)]
# packed chunk: y rows [250, 256), X rows 248..255 + dup, 9 partitions per t
C2_M0 = 250
C2_R0 = 248
C2_TILES = [(0, 14), (14, 2)]  # (t0, nt)
NMAT_PER_CHUNK = 30  # per c: 5 avg + corr0 + corr255 + 3 max
NMATS = 90

_cache = {}


def _build_ops(conv_w):
    """Per-channel operator matrices in float64.

    Returns per c: (OP5[5], corr0, corr255, OPmax[3]) each HxH acting on rows:
      y[m, w] = sum_s OP5_s[m, :] @ X[:, w+s-2]  (+ corr at w=0 / w=255)
             + sum_kw OPmax_kw[m, :] @ MX[:, w+kw-1]
    """
    w = conv_w[0].astype(np.float64)  # [6, 3, 3]

    def S(j):
        return np.eye(H, k=j)

    Bv = S(-1) + S(0) + S(1)
    out = []
    for c in range(3):
        wM, wA = w[2 * c], w[2 * c + 1]
        OPmax = [sum(wM[kh, kw] * S(kh - 1) for kh in range(3)) for kw in range(3)]
        OPv = [(sum(wA[kh, kw] * S(kh - 1) for kh in range(3)) @ Bv) / 9.0
               for kw in range(3)]
        OP5 = [np.zeros((H, H)) for _ in range(5)]
        for kw in range(3):
            for j in range(3):
                OP5[kw + j] += OPv[kw]
        out.append((OP5, -OPv[0], -OPv[2], OPmax))
    return out


def _mat_list(ops_c):
    """Order mats per c: OP5 s=0..4, corr0, corr255, OPmax kw=0..2 (10 mats)."""
    OP5, corr0, corr255, OPmax = ops_c
    return list(OP5) + [corr0, corr255] + list(OPmax)


def _build_stack(conv_w):
    """lhsT stack [128, NMATS, 128] bf16.

    mats 0..29:  chunk0, lhsT[j, mm] = MAT[m0+mm, r0+j]
    mats 30..59: chunk1, same
    mats 60..89: packed chunk, block-diagonal over 14 t:
                 lhsT[9t+j, 6t+mm] = MAT[250+mm, 248+j] (j=8 -> 0)
    """
    ops = _build_ops(conv_w)
    stack = np.zeros((128, NMATS, 128), dtype=np.float64)
    for g, (m0, m1, r0) in enumerate(BIG_CHUNKS):
        M = m1 - m0
        for c in range(3):
            for k, MAT in enumerate(_mat_list(ops[c])):
                stack[:, 30 * g + 10 * c + k, :M] = MAT[m0:m1, r0:r0 + 128].T
    for c in range(3):
        for k, MAT in enumerate(_mat_list(ops[c])):
            m = np.zeros((126, 84))
            for t in range(14):
                for j in range(8):  # j=8 (dup/pooled-256) stays 0
                    m[9 * t + j, 6 * t:6 * t + 6] = MAT[C2_M0:C2_M0 + 6,
                                                        C2_R0 + j]
            stack[:126, 60 + 10 * c + k, :84] = m
    return stack.astype(ml_dtypes.bfloat16)


# (kind, shift): kinds 'x' (rhs = X) and 'mx' (rhs = MX); corr handled apart
MAT_SHIFTS = [('x', s - 2) for s in range(5)] + [('corr0', 0), ('corr255', 0)] \
    + [('mx', kw - 1) for kw in range(3)]


def _build_program():
    nc = bacc.Bacc("TRN2", target_bir_lowering=False, debug=False,
                   enable_asserts=False)
    # host-pre-transposed bf16 input: x_rm[r, ch, w]
    x_ap = nc.dram_tensor("x", [H, CTOT, W], BF16, kind="ExternalInput").ap()
    cst_ap = nc.dram_tensor("cst", [128, NMATS, 128], BF16,
                            kind="ExternalInput").ap()
    bias_ap = nc.dram_tensor("bias", [128, 1], F32, kind="ExternalInput").ap()
    # row-major scratch: outm[r, ch, w]; host transposes to [ch, r, w]
    outm_ap = nc.dram_tensor("outm", [H, CTOT, W], BF16,
                             kind="ExternalOutput").ap()
    out2_aps = [nc.dram_tensor(f"out2{i}", [6 * nt, 3, W], BF16,
                               kind="ExternalOutput").ap()
                for i, (t0, nt) in enumerate(C2_TILES)]

    MAXOP = mybir.AluOpType.max
    LRELU = mybir.ActivationFunctionType.Lrelu
    SIGM = mybir.ActivationFunctionType.Sigmoid

    with tile.TileContext(nc) as tc, ExitStack() as ctx:
        const_pool = ctx.enter_context(tc.tile_pool(name="const", bufs=1))
        xpool = ctx.enter_context(tc.tile_pool(name="xp", bufs=3))
        mxpool = ctx.enter_context(tc.tile_pool(name="mxp", bufs=2))
        p1pool = ctx.enter_context(tc.tile_pool(name="p1", bufs=2))
        bigpool = ctx.enter_context(tc.tile_pool(name="bg", bufs=1))
        epipool = ctx.enter_context(tc.tile_pool(name="epi", bufs=8))
        psum_pool = ctx.enter_context(tc.tile_pool(name="psum", bufs=8,
                                                   space="PSUM"))

        cst = const_pool.tile([128, NMATS, 128], BF16, tag="cst")
        nc.gpsimd.dma_start(out=cst[:], in_=cst_ap)
        bias = const_pool.tile([128, 1], F32, tag="bias")
        nc.sync.dma_start(out=bias[:], in_=bias_ap)

        def hpool(vmax, np_, fd):
            """Horizontal 3-max of vmax -> mx, cols in-lane."""
            hb = p1pool.tile([128, PGRP, W], BF16, tag="hb")
            nc.vector.tensor_tensor(out=hb[0:np_, 0:fd, 0:W - 2],
                                    in0=vmax[0:np_, 0:fd, 0:W - 2],
                                    in1=vmax[0:np_, 0:fd, 2:W], op=MAXOP)
            mx = mxpool.tile([128, PGRP, W], BF16, tag="mx")
            nc.vector.tensor_tensor(out=mx[0:np_, 0:fd, 1:W - 1],
                                    in0=vmax[0:np_, 0:fd, 1:W - 1],
                                    in1=hb[0:np_, 0:fd, 0:W - 2], op=MAXOP)
            nc.vector.tensor_tensor(out=mx[0:np_, 0:fd, 0:1],
                                    in0=vmax[0:np_, 0:fd, 0:1],
                                    in1=vmax[0:np_, 0:fd, 1:2], op=MAXOP)
            nc.vector.tensor_tensor(out=mx[0:np_, 0:fd, W - 1:W],
                                    in0=vmax[0:np_, 0:fd, W - 2:W - 1],
                                    in1=vmax[0:np_, 0:fd, W - 1:W], op=MAXOP)
            return mx

        import os as _os0
        DBG_NH = int(_os0.environ.get("DBG_NH", "16"))    # groups to emit
        DBG_NOMM = _os0.environ.get("DBG_NOMM", "0") == "1"
        DBG_NOEPI = _os0.environ.get("DBG_NOEPI", "0") == "1"
        half_ctr = [0]

        # ---------------- big chunks ----------------
        def big_chunk(g):
            m0, m1, r0 = BIG_CHUNKS[g]
            M = m1 - m0
            big = bigpool.tile([128, CTOT, W], BF16, tag="big")
            epi_jobs = []
            for h in range(CTOT // PGRP):
                if half_ctr[0] >= DBG_NH:
                    break
                half_ctr[0] += 1
                pl = slice(PGRP * h, PGRP * (h + 1))
                # row-major bf16 input: every load is partition-contiguous
                # (1 descriptor per partition) - SWDGE issue is trivial
                X = xpool.tile([128, PGRP, W], BF16, tag="x")
                nc.gpsimd.dma_start(out=X[:], in_=x_ap[r0:r0 + 128, pl, :])
                U = p1pool.tile([128, PGRP, W], BF16, tag="u")
                nc.gpsimd.dma_start(out=U[:], in_=x_ap[r0 + 1:r0 + 129, pl, :])
                Df = p1pool.tile([128, PGRP, W], BF16, tag="df")
                if r0 == 0:
                    nc.gpsimd.dma_start(out=Df[1:128], in_=x_ap[0:127, pl, :])
                    nc.gpsimd.dma_start(out=Df[0:1], in_=x_ap[0:1, pl, :])
                else:
                    nc.gpsimd.dma_start(out=Df[:],
                                        in_=x_ap[r0 - 1:r0 + 127, pl, :])
                m2 = p1pool.tile([128, PGRP, W], BF16, tag="m2")
                nc.vector.tensor_tensor(out=m2[:], in0=X[:], in1=U[:],
                                        op=MAXOP)
                vmax = p1pool.tile([128, PGRP, W], BF16, tag="vm")
                nc.vector.tensor_tensor(out=vmax[:], in0=m2[:], in1=Df[:],
                                        op=MAXOP)
                mx = hpool(vmax, 128, PGRP)

                pairs = range(2 * h, 2 * h + 2)
                ps_tiles = {}
                for tp in pairs:
                    ps_tiles[tp] = psum_pool.tile([128, 2, W], F32, tag="ps",
                                                  name=f"ps_{g}_{tp}")

                if DBG_NOMM:
                    continue
                # emission: X-dependent mats first (PE can start before
                # pools finish), then mx-dependent; mat-outer, pair-inner
                order = [(c, k) for c in range(3) for k in range(7)] + \
                    [(c, k) for c in range(3) for k in range(7, 10)]
                for mi, (c, k) in enumerate(order):
                    kind, s = MAT_SHIFTS[k]
                    if True:
                        gmat = 30 * g + 10 * c + k
                        first = (mi == 0)
                        last = (mi == NMAT_PER_CHUNK - 1)
                        lhsT = cst[0:128, gmat, 0:M]
                        for tp in pairs:
                            ps = ps_tiles[tp]
                            lq = 6 * tp + c - PGRP * h
                            if kind == 'corr0':
                                nc.tensor.matmul(
                                    ps[0:M, :, 0:1], lhsT,
                                    X[0:128, lq:lq + 4:3, 0:1],
                                    start=first, stop=last)
                            elif kind == 'corr255':
                                nc.tensor.matmul(
                                    ps[0:M, :, W - 1:W], lhsT,
                                    X[0:128, lq:lq + 4:3, W - 1:W],
                                    start=first, stop=last)
                            else:
                                src = X if kind == 'x' else mx
                                lo = max(0, -s)
                                hi = W - max(0, s)
                                nc.tensor.matmul(
                                    ps[0:M, :, lo:hi], lhsT,
                                    src[0:128, lq:lq + 4:3, lo + s:hi + s],
                                    start=first, stop=last)
                epi_jobs.append((list(pairs), ps_tiles))

            import os as _os
            if DBG_NOEPI:
                epi_jobs.clear()
            if _os.environ.get("HALF_EPI", "1") == "1":
                for hh, (pairs_l, ps_tiles) in enumerate(epi_jobs):
                    v_tiles = {}
                    for tp in pairs_l:
                        v = epipool.tile([128, 2, W], BF16, tag="v",
                                         name=f"v_{g}_{tp}")
                        nc.scalar.activation(v[0:M], ps_tiles[tp][0:M], LRELU,
                                             bias=bias[0:M], scale=1.0,
                                             alpha=0.01)
                        v_tiles[tp] = v
                    for tp in pairs_l:
                        for c3 in range(3):
                            q = 6 * tp + c3
                            nc.scalar.activation(big[0:M, q:q + 4:3, :],
                                                 v_tiles[tp][0:M], SIGM)
                    pl0 = PGRP * hh
                    nc.gpsimd.dma_start(
                        out=outm_ap[m0:m1, pl0:pl0 + PGRP, :],
                        in_=big[0:M, pl0:pl0 + PGRP, :])
                return
            if _os.environ.get("NOEPI_BATCH", "0") == "1":
                for pairs_l, ps_tiles in epi_jobs:
                    for tp in pairs_l:
                        v = epipool.tile([128, 2, W], BF16, tag="v",
                                         name=f"v_{g}_{tp}")
                        nc.scalar.activation(v[0:M], ps_tiles[tp][0:M], LRELU,
                                             bias=bias[0:M], scale=1.0,
                                             alpha=0.01)
                        for c3 in range(3):
                            q = 6 * tp + c3
                            nc.scalar.activation(big[0:M, q:q + 4:3, :],
                                                 v[0:M], SIGM)
            else:
                v_tiles = {}
                for pairs_l, ps_tiles in epi_jobs:
                    for tp in pairs_l:
                        v = epipool.tile([128, 2, W], BF16, tag="v",
                                         name=f"v_{g}_{tp}")
                        nc.scalar.activation(v[0:M], ps_tiles[tp][0:M], LRELU,
                                             bias=bias[0:M], scale=1.0,
                                             alpha=0.01)
                        v_tiles[tp] = v
                for pairs_l, _ in epi_jobs:
                    for tp in pairs_l:
                        for c3 in range(3):
                            q = 6 * tp + c3
                            nc.scalar.activation(big[0:M, q:q + 4:3, :],
                                                 v_tiles[tp][0:M], SIGM)
            if _os.environ.get("NOOUT", "0") != "1" and not DBG_NOEPI:
                nc.gpsimd.dma_start(out=outm_ap[m0:m1], in_=big[0:M])

        # ---------------- packed last-rows chunk ----------------
        def packed_tile(i):
            t0, nt = C2_TILES[i]
            npart = 9 * nt
            X2 = xpool.tile([128, 3, W], BF16, tag="x2")
            # finite-fill the tail partitions (quadrant-aligned base) before
            # the real loads overwrite [0:npart)
            nc.vector.tensor_copy(
                X2[:].rearrange("p c w -> p (c w)"),
                cst[0:128, 0:6, 0:128].rearrange("p a b -> p (a b)"))
            for t in range(nt):
                nc.gpsimd.dma_start(
                    out=X2[9 * t:9 * t + 8],
                    in_=x_ap[C2_R0:C2_R0 + 8,
                             3 * (t0 + t):3 * (t0 + t) + 3, :])
            dup = x_ap[H - 1, 3 * t0:3 * (t0 + nt), :].rearrange(
                "(t c) w -> t c w", t=nt)
            nc.gpsimd.dma_start(
                out=X2[0:126].rearrange("(t r) c w -> t r c w",
                                        t=14)[0:nt, 8:9].squeeze(1),
                in_=dup)
            
            Xs = p1pool.tile([128, PGRP, W], BF16, tag="hb")  # reuse slot
            nc.sync.dma_start(out=Xs[0:npart, 0:3], in_=X2[1:npart + 1])
            m2 = p1pool.tile([128, PGRP, W], BF16, tag="m2")
            nc.vector.tensor_tensor(out=m2[0:npart, 0:3], in0=X2[0:npart],
                                    in1=Xs[0:npart, 0:3], op=MAXOP)
            m2s = p1pool.tile([128, PGRP, W], BF16, tag="df")  # reuse slot
            nc.sync.dma_start(out=m2s[1:npart, 0:3], in_=m2[0:npart - 1, 0:3])
            nc.vector.tensor_copy(m2s[0:1, 0:3], m2[0:1, 0:3])
            vmax = p1pool.tile([128, PGRP, W], BF16, tag="vm")
            nc.vector.tensor_tensor(out=vmax[0:npart, 0:3], in0=m2[0:npart, 0:3],
                                    in1=m2s[0:npart, 0:3], op=MAXOP)
            mx2 = hpool(vmax, npart, 3)

            ps = psum_pool.tile([128, 2, W], F32, tag="ps", name=f"ps2_{i}")
            mi = 0
            for c in range(3):
                for k, (kind, s) in enumerate(MAT_SHIFTS):
                    gmat = 60 + 10 * c + k
                    lhsT = cst[0:npart, gmat, 0:6 * nt]
                    first = (mi == 0)
                    last = (mi == NMAT_PER_CHUNK - 1)
                    if kind == 'corr0':
                        nc.tensor.matmul(ps[0:6 * nt, 0, 0:1], lhsT,
                                         X2[0:npart, c, 0:1],
                                         start=first, stop=last)
                    elif kind == 'corr255':
                        nc.tensor.matmul(ps[0:6 * nt, 0, W - 1:W], lhsT,
                                         X2[0:npart, c, W - 1:W],
                                         start=first, stop=last)
                    else:
                        src = X2 if kind == 'x' else mx2
                        lo = max(0, -s)
                        hi = W - max(0, s)
                        nc.tensor.matmul(ps[0:6 * nt, 0, lo:hi], lhsT,
                                         src[0:npart, c, lo + s:hi + s],
                                         start=first, stop=last)
                    mi += 1

            v = epipool.tile([128, 2, W], BF16, tag="v", name=f"v2_{i}")
            nc.scalar.activation(v[0:6 * nt, 0], ps[0:6 * nt, 0], LRELU,
                                 bias=bias[0:6 * nt], scale=1.0, alpha=0.01)
            big2 = bigpool.tile([128, 3, W], BF16, tag="big2")
            for c3 in range(3):
                nc.scalar.activation(big2[0:6 * nt, c3, :], v[0:6 * nt, 0], SIGM)
            nc.gpsimd.dma_start(out=out2_aps[i], in_=big2[0:6 * nt])

        import os
        if os.environ.get("C2MID", "1") == "1":
            big_chunk(0)
            packed_tile(0)
            packed_tile(1)
            big_chunk(1)
        else:
            import os as _os
            big_chunk(0)
            big_chunk(1)
            if _os.environ.get("SKIP_C2", "0") != "1":
                packed_tile(0)
                packed_tile(1)

    nc.compile()
    return nc


def _assemble(res):
    outs = []
    for i in range(N_CORES):
        om = np.asarray(res.results[i]["outm"]).astype(np.float32)
        o = np.ascontiguousarray(om.transpose(1, 0, 2))  # [ch, r, w]
        for ti, (t0, nt) in enumerate(C2_TILES):
            o2 = np.asarray(res.results[i][f"out2{ti}"]).astype(np.float32)
            for t in range(nt):
                for c3 in range(3):
                    o[3 * (t0 + t) + c3, C2_M0:C2_M0 + 6, :] = \
                        o2[6 * t:6 * t + 6, c3]
        outs.append(o)
    return np.stack(outs, axis=0)


def _prep_x(input_tensor):
    """[B, CTOT, H, W] f32 -> per-core row-major bf16 [H, CTOT, W]."""
    xb = np.asarray(input_tensor, dtype=np.float32).astype(ml_dtypes.bfloat16)
    xrm = np.ascontiguousarray(xb.transpose(0, 2, 1, 3))
    return xrm


def kernel(input_tensor, conv_w, conv_b):
    conv_w = np.asarray(conv_w, dtype=np.float32)
    conv_b = np.asarray(conv_b, dtype=np.float32)

    if "nc" not in _cache:
        _cache["nc"] = _build_program()
    nc = _cache["nc"]

    xrm = _prep_x(input_tensor)
    stack = _build_stack(conv_w)
    bias_vec = np.full((128, 1), conv_b[0], dtype=np.float32)
    in_maps = [
        {"x": xrm[i], "cst": stack, "bias": bias_vec}
        for i in range(N_CORES)
    ]
    res = run_bass_kernel_spmd(nc, in_maps, list(range(N_CORES)))
    return _assemble(res)


if __name__ == "__main__":
    rng = np.random.default_rng(0)
    x = rng.standard_normal((B, CTOT, H, W), dtype=np.float32)
    cw = rng.uniform(-0.1, 0.1, (1, 6, 3, 3)).astype(np.float32)
    cb = np.array([0.01], dtype=np.float32)
    o = kernel(x, cw, cb)
    print(o.shape, o.dtype)
